# revision 13
# baseline (speedup 1.0000x reference)
"""Quantized linear (dynamic per-tensor int8) on 8 TRN2 NeuronCores.

Reference semantics:
    x_q = round(x / s_x), s_x = max|x|/127   (per-tensor, round-half-even)
    w_q = round(w / s_w), s_w = max|w|/127
    out = (x_q @ w_q.T) * (s_x * s_w) + bias

Distribution: data-parallel over M (8 shards of 1024 rows), weight
replicated.  Each core scans a disjoint 1/8 of x (its own shard) and of w
for the local absmax; a 2-element AllReduce(max) collective produces the
global scales.  Quantized values are exact small integers, held in bf16
(ints <= 127 are exact in bf16), so the TensorE bf16 matmul with fp32 PSUM
accumulation reproduces the int8 GEMM exactly (sums stay far below 2^24).

Rounding uses the fp32 magic-number trick: RNE(round(v)) == (v + 1.5*2^23)
- 1.5*2^23 for |v| <= 2^22, matching jnp.round (half-to-even).

Host-side work is layout only: inputs are passed transposed (K-major) so
both matmul operands land in SBUF with K on the partition axis without any
on-device transposes; the output is computed as out^T (N on partitions) so
the bias add is a per-partition ScalarE bias, and the host transposes back.
"""

import numpy as np

from concourse import bacc, bass_isa
import concourse.bass_utils as bass_utils
import concourse.mybir as mybir
import concourse.tile as tile

P = 128
M, K, N = 8192, 4096, 4096
NCORES = 8
MLOC = M // NCORES  # 1024 rows of x per core
WS = N // NCORES  # 512 columns of wT scanned per core for absmax
MAGIC = float(np.float32(1.5 * 2**23))
MFREE = 512  # moving free dim per matmul (one fp32 PSUM bank)
NSTRIP = 128  # n-columns of w quantized per strip

F32 = mybir.dt.float32
BF16 = mybir.dt.bfloat16
AX = mybir.AxisListType
ALU = mybir.AluOpType
ACTF = mybir.ActivationFunctionType


def build_body(tc, xT, wT, wscanT, bias, outT, *, n_cores, mfree, nstrip):
    nc = tc.nc
    k, m_loc = xT.shape
    n = wT.shape[1]
    ws = wscanT.shape[1]
    kt_n = k // P
    assert k % P == 0 and n % nstrip == 0 and nstrip % P == 0 and m_loc % mfree == 0

    with (
        tc.tile_pool(name="const", bufs=1) as const,
        tc.tile_pool(name="stats", bufs=1) as stats,
        tc.tile_pool(name="xf", bufs=2) as xf_pool,
        tc.tile_pool(name="xfq", bufs=2) as xfq_pool,
        tc.tile_pool(name="xq", bufs=1) as xq_pool,
        tc.tile_pool(name="wf", bufs=2) as wf_pool,
        tc.tile_pool(name="wq", bufs=4) as wq_pool,
        tc.tile_pool(name="ob", bufs=4) as ob_pool,
        tc.tile_pool(name="ps", bufs=6, space="PSUM") as ps_pool,
        tc.tile_pool(name="dram", bufs=1, space="DRAM") as dram,
    ):
        # ---- bias, laid out bias[j*128+p] -> bias_sb[p, j] --------------
        bias_sb = const.tile([P, n // P], F32)
        nc.sync.dma_start(bias_sb[:], bias.rearrange("(nt p) -> p nt", p=P))

        xT3 = xT.rearrange("(c p) m -> p c m", p=P)
        wsT3 = wscanT.rearrange("(c p) m -> p c m", p=P)
        wT3 = wT.rearrange("(kt p) n -> p kt n", p=P)

        # ---- phase A: absmax scans.  w first: its collective result is the
        # first thing the matmul supply chain (w quantize) needs.
        WCK, XCK = 4, 2  # k-tiles per scan chunk
        n_wsc, n_xsc = kt_n // WCK, kt_n // XCK
        wmax_cols = stats.tile([P, n_wsc], F32)
        xmax_cols = stats.tile([P, n_xsc], F32)
        for i in range(n_wsc):
            tw = xf_pool.tile([P, WCK, ws], F32, tag="wscan")
            nc.sync.dma_start(tw[:], wsT3[:, i * WCK : (i + 1) * WCK, :])
            nc.vector.tensor_reduce(
                wmax_cols[:, i : i + 1], tw[:], axis=AX.XY, op=ALU.max,
                apply_absolute_value=True,
            )
        wlmax = stats.tile([P, 1], F32)
        nc.vector.tensor_reduce(wlmax[:], wmax_cols[:], axis=AX.X, op=ALU.max)
        wgmax_p = stats.tile([P, 1], F32)
        nc.gpsimd.partition_all_reduce(
            wgmax_p[:], wlmax[:], channels=P, reduce_op=bass_isa.ReduceOp.max
        )
        wcc_in = dram.tile([1, 1], F32)
        wcc_out = dram.tile([1, 1], F32)
        nc.gpsimd.dma_start(wcc_in[:], wgmax_p[0:1, :])
        nc.gpsimd.collective_compute(
            "AllReduce", ALU.max, replica_groups=[list(range(n_cores))],
            ins=[wcc_in.opt()], outs=[wcc_out.opt()],
        )
        # x scan (concurrent with the w collective round-trip)
        for i in range(n_xsc):
            t = xf_pool.tile([P, XCK, m_loc], F32, tag="xf")
            nc.sync.dma_start(t[:], xT3[:, i * XCK : (i + 1) * XCK, :])
            nc.vector.tensor_reduce(
                xmax_cols[:, i : i + 1], t[:], axis=AX.XY, op=ALU.max,
                apply_absolute_value=True,
            )
        xlmax = stats.tile([P, 1], F32)
        nc.vector.tensor_reduce(xlmax[:], xmax_cols[:], axis=AX.X, op=ALU.max)
        xgmax_p = stats.tile([P, 1], F32)
        nc.gpsimd.partition_all_reduce(
            xgmax_p[:], xlmax[:], channels=P, reduce_op=bass_isa.ReduceOp.max
        )
        xcc_in = dram.tile([1, 1], F32)
        xcc_out = dram.tile([1, 1], F32)
        nc.gpsimd.dma_start(xcc_in[:], xgmax_p[0:1, :])
        nc.gpsimd.collective_compute(
            "AllReduce", ALU.max, replica_groups=[list(range(n_cores))],
            ins=[xcc_in.opt()], outs=[xcc_out.opt()],
        )

        wgmax = stats.tile([1, 1], F32)
        nc.gpsimd.dma_start(wgmax[:], wcc_out[:])
        # w scales: inv_sw = 127/wmax (quantize); s_w = wmax/127 (dequant)
        wsc2 = stats.tile([1, 2], F32)
        wrec = stats.tile([1, 1], F32)
        nc.vector.reciprocal(wrec[:], wgmax[:])
        nc.vector.tensor_scalar(wsc2[:, 0:1], wrec[:], 127.0, None, op0=ALU.mult)
        nc.vector.tensor_scalar(
            wsc2[:, 1:2], wgmax[:], float(np.float32(1.0 / 127.0)), None,
            op0=ALU.mult,
        )
        wscb = const.tile([P, 2], F32)
        nc.gpsimd.partition_broadcast(wscb[:], wsc2[:])
        inv_sw = wscb[:, 0:1]
        s_w = wscb[:, 1:2]


        # ---- pre-quantize the first strips of w (ACT) so the PE supply
        # chain never sits behind x work in any engine FIFO
        n_strips = n // nstrip
        pre_q = min(4, n_strips)
        wqs = {}
        for s in range(pre_q):
            wf = wf_pool.tile([P, kt_n, nstrip], F32, tag="wf")
            nc.sync.dma_start(wf[:], wT3[:, :, s * nstrip : (s + 1) * nstrip])
            nc.scalar.activation(wf[:], wf[:], ACTF.Copy, bias=MAGIC, scale=inv_sw)
            wq = wq_pool.tile([P, kt_n, nstrip], BF16, tag="wq")
            nc.vector.tensor_scalar(wq[:], wf[:], MAGIC, None, op0=ALU.subtract)
            wqs[s] = wq

        xgmax = stats.tile([1, 1], F32)
        nc.gpsimd.dma_start(xgmax[:], xcc_out[:])
        # x scales: inv_sx = 127/xmax; out_scale = s_x * s_w
        xsc2 = stats.tile([1, 2], F32)
        xrec = stats.tile([1, 1], F32)
        nc.vector.reciprocal(xrec[:], xgmax[:])
        nc.vector.tensor_scalar(xsc2[:, 0:1], xrec[:], 127.0, None, op0=ALU.mult)
        nc.vector.tensor_scalar(
            xsc2[:, 1:2], xgmax[:], float(np.float32(1.0 / 127.0)), None,
            op0=ALU.mult,
        )
        xscb = const.tile([P, 2], F32)
        nc.gpsimd.partition_broadcast(xscb[:], xsc2[:])
        inv_sx = xscb[:, 0:1]
        out_sc = const.tile([P, 1], F32)
        nc.vector.tensor_tensor(out_sc[:], xscb[:, 1:2], s_w, op=ALU.mult)

        # ---- phase C: quantize x shard -> resident bf16 (all on DVE) ----
        n_mh = m_loc // mfree
        QCK = 4  # k-tiles per quantize chunk
        xqs = [xq_pool.tile([P, kt_n, mfree], BF16, name=f"xq{h}") for h in range(n_mh)]
        for h in range(n_mh):
            for i in range(kt_n // QCK):
                xf = xfq_pool.tile([P, QCK, mfree], F32, tag="xfq")
                nc.sync.dma_start(
                    xf[:],
                    xT3[:, i * QCK : (i + 1) * QCK, h * mfree : (h + 1) * mfree],
                )
                nc.vector.tensor_scalar(
                    xf[:], xf[:], inv_sx, MAGIC, op0=ALU.mult, op1=ALU.add
                )
                nc.vector.tensor_scalar(
                    xqs[h][:, i * QCK : (i + 1) * QCK, :], xf[:], MAGIC, None,
                    op0=ALU.subtract,
                )

        # ---- phase D: stream w strips, quantize (ACT), matmul, evict ----
        for s in range(n_strips):
            if s in wqs:
                wq = wqs[s]
            else:
                wf = wf_pool.tile([P, kt_n, nstrip], F32, tag="wf")
                nc.sync.dma_start(wf[:], wT3[:, :, s * nstrip : (s + 1) * nstrip])
                nc.scalar.activation(
                    wf[:], wf[:], ACTF.Copy, bias=MAGIC, scale=inv_sw
                )
                wq = wq_pool.tile([P, kt_n, nstrip], BF16, tag="wq")
                nc.vector.tensor_scalar(wq[:], wf[:], MAGIC, None, op0=ALU.subtract)
            for nt in range(nstrip // P):
                gn = s * nstrip + nt * P  # global n of this out^T row-tile
                for mh in range(m_loc // mfree):
                    ps = ps_pool.tile([P, mfree], F32)
                    for kt in range(kt_n):
                        nc.tensor.matmul(
                            ps[:],
                            wq[:, kt, nt * P : (nt + 1) * P],
                            xqs[mh][:, kt, :],
                            start=(kt == 0),
                            stop=(kt == kt_n - 1),
                        )
                    ob = ob_pool.tile([P, mfree], F32, tag="ob")
                    nc.vector.tensor_scalar(
                        ob[:], ps[:], out_sc[:], bias_sb[:, gn // P : gn // P + 1],
                        op0=ALU.mult, op1=ALU.add,
                    )
                    nc.gpsimd.dma_start(
                        outT[gn : gn + P, mh * mfree : (mh + 1) * mfree], ob[:]
                    )


def build_nc(m_loc=MLOC, k=K, n=N, ws=WS, n_cores=NCORES, mfree=MFREE, nstrip=NSTRIP):
    nc = bacc.Bacc("TRN2", target_bir_lowering=False, debug=False,
                   num_devices=n_cores)
    xT = nc.dram_tensor("xT", [k, m_loc], F32, kind="ExternalInput").ap()
    wT = nc.dram_tensor("wT", [k, n], F32, kind="ExternalInput").ap()
    wscanT = nc.dram_tensor("wscanT", [k, ws], F32, kind="ExternalInput").ap()
    bias = nc.dram_tensor("bias", [n], F32, kind="ExternalInput").ap()
    outT = nc.dram_tensor("outT", [n, m_loc], F32, kind="ExternalOutput").ap()
    with tile.TileContext(nc) as tc:
        build_body(tc, xT, wT, wscanT, bias, outT,
                   n_cores=n_cores, mfree=mfree, nstrip=nstrip)
    nc.compile()
    return nc


def make_in_maps(x, weight, bias, n_cores=NCORES):
    m_loc = x.shape[0] // n_cores
    ws = weight.shape[0] // n_cores
    wT = np.ascontiguousarray(weight.T)
    bias = np.ascontiguousarray(bias, dtype=np.float32)
    maps = []
    for c in range(n_cores):
        maps.append({
            "xT": np.ascontiguousarray(x[c * m_loc : (c + 1) * m_loc].T),
            "wT": wT,
            "wscanT": np.ascontiguousarray(weight[c * ws : (c + 1) * ws].T),
            "bias": bias,
        })
    return maps


_NC_CACHE = {}
LAST_RUN = None


def kernel(x, weight, bias, _trace=False):
    global LAST_RUN
    x = np.ascontiguousarray(np.asarray(x), dtype=np.float32)
    weight = np.ascontiguousarray(np.asarray(weight), dtype=np.float32)
    bias = np.asarray(bias, dtype=np.float32)
    if "full" not in _NC_CACHE:
        _NC_CACHE["full"] = build_nc()
    nc = _NC_CACHE["full"]
    in_maps = make_in_maps(x, weight, bias)
    res = bass_utils.run_bass_kernel_spmd(
        nc, in_maps, core_ids=list(range(NCORES)), trace=_trace
    )
    LAST_RUN = res
    out = np.empty((M, N), np.float32)
    for c in range(NCORES):
        out[c * MLOC : (c + 1) * MLOC, :] = res.results[c]["outT"].T
    return out
